# revision 35
# baseline (speedup 1.0000x reference)
"""DIN attention layer kernel for Trainium2 - batch-PAIR token packing.

Per batch b (reference): att=[q,k,q-k,q*k]; h1=relu(att@W1+b1);
h2=relu(h1@W2+b2); s=h2@w_score; attn=softmax(s + mask*-1e9);
out=attn@values.

Optimizations:
  * Host token compaction: masked tokens (mask==1 -> -1e9 logit -> attn
    weight exactly 0 in fp32) are dropped on the host; only the ~50%
    real tokens reach the device.
  * Batch-pair packing: two batches' real tokens share one packed token
    axis of ceil((LA+LB)/128) chunks (LA/LB = per-slot maxima over
    cores, baked). mm2 + score work are batch-agnostic; pad waste drops
    from ~132 tokens/batch to ~64/pair (40 -> ~33 chunks/core). The
    A|B boundary chunk is handled with 0/1 column selectors (SEL) so no
    partition-offset matmuls are needed; per-batch softmax sums come
    from splitting the Exp activation at the boundary + a DVE add.
  * Concat-matmul reassociation: att@W1 = q@(W1a+W1c) [host, folds into
    the rt bias] + k@[(W1b-W1c) + diag(q)W1d] [device mm1, contraction
    256]. The per-batch W1eff is computed on the host and DMA'd.
  * All tensors arrive in exact SBUF tile layout ([P, ...] contiguous
    per partition) so every DMA is a handful of multi-KB descriptors.
  * mm2 in transposed-output form; score falls out of the PSUM drain
    via relu-accumulate with W2 columns pre-permuted by sign(w_score)
    and pre-scaled by |w_score|. Each 512-col half has its own PSUM
    pool; half 0 accumulates on the DVE, half 1 on Scalar.
  * mm2 hybrid precision: 6 of 8 contraction chunks fp8e4 DoubleRow
    (2x), 2 bf16; rel err ~1.77e-2 vs the 2e-2 gate.
  * Softmax without max-subtraction; attn@values accumulated per 128-
    token chunk with exp as lhsT (scores land partition-striped free).
    1/sum is partition-broadcast via a ones-matrix matmul and folded
    into exp BEFORE attn@values, so the PSUM result is final (just a
    scalar-engine copy + DMA, no serial 1-partition multiplies).
    For carried pairs both batches share one 2-column scaled-exp tile,
    so each values chunk is streamed once for both outputs.
  * Software pipelining: pair s emits mm1(s), attn(s-1), mm2(s); the
    last pair splits its softmax so batch A's attn chain overlaps the
    tail chunks of mm2.
"""

import os
import numpy as np

P = 128
B = 8          # batches per core
NPAIR = 4      # batch pairs per core
T = 1024       # tokens (full, pre-compaction)
M = 256        # key feature dim
D = 1024       # hidden dim
MC = M // P    # key-feature chunks (2)
DC = D // P    # hidden chunks (8)
NH = 2         # free-dim halves of 512
NEG = -1.0e9
S_W2 = 512.0   # pre-scale on W2'' (keeps fp8 path out of denormals)
FP8K = int(os.environ.get("DIN_FP8K", "6"))   # mm2 contraction chunks in fp8
BFK = DC - FP8K

_built = {}


def _ns(h):
    return slice(h * 512, (h + 1) * 512)


def _segs(a, b):
    """Split [a, b) into free-dim segments of <= 512."""
    return [(s, min(s + 512, b)) for s in range(a, b, 512)]


def _build(n_pos, params):
    import concourse.bass as bass
    import concourse.bacc as bacc
    import concourse.mybir as mybir
    import concourse.tile as tile
    from contextlib import ExitStack

    F32 = mybir.dt.float32
    BF16 = mybir.dt.bfloat16
    FP8 = mybir.dt.float8e4
    AF = mybir.ActivationFunctionType
    OP = mybir.AluOpType
    DR = mybir.MatmulPerfMode.DoubleRow

    geo = []
    for (LA, LB) in params:
        L2 = LA + LB
        TCp = -(-L2 // P)
        cb, rb = divmod(LA, P)
        assert cb >= 1 and TCp - cb >= 2, (LA, LB)
        geo.append((LA, LB, L2, TCp, cb, rb))
    TCmax = max(g[3] for g in geo)
    TCp0 = geo[0][3]
    Tp0 = TCp0 * P
    sA0 = min(512, Tp0)

    nc = bacc.Bacc("TRN2")
    # pair 0's X / W1eff arrive as split tensors for a fast start
    x0a_d = nc.dram_tensor("X0A", [P, MC, sA0], BF16, kind="ExternalInput").ap()
    x0b_d = nc.dram_tensor("X0B", [P, MC, Tp0 - sA0], BF16,
                           kind="ExternalInput").ap()
    x_ds = [None] + [nc.dram_tensor(f"X{s}", [P, MC, geo[s][3] * P], BF16,
                                    kind="ExternalInput").ap()
                     for s in range(1, NPAIR)]
    v_ds = [nc.dram_tensor(f"V{s}", [P, geo[s][3], D], BF16,
                           kind="ExternalInput").ap() for s in range(NPAIR)]
    rt_d = nc.dram_tensor("RT", [P, B, DC], F32, kind="ExternalInput").ap()
    mn_d = nc.dram_tensor("MASKN", [P, NPAIR, TCmax], F32, kind="ExternalInput").ap()
    sel_d = nc.dram_tensor("SEL", [P, NPAIR, 2], BF16, kind="ExternalInput").ap()
    we0a_d = nc.dram_tensor("WE0A", [P, MC, D // 2], BF16, kind="ExternalInput").ap()
    we0b_d = nc.dram_tensor("WE0B", [P, MC, D // 2], BF16, kind="ExternalInput").ap()
    w1e_d = nc.dram_tensor("W1EFF", [B, P, MC, D], BF16, kind="ExternalInput").ap()
    w2q_d = (nc.dram_tensor("W2Q", [P, FP8K, D], FP8, kind="ExternalInput").ap()
             if FP8K > 0 else None)
    w2b_d = (nc.dram_tensor("W2B", [P, BFK, D], BF16, kind="ExternalInput").ap()
             if BFK > 0 else None)
    out_d = nc.dram_tensor("out", [B, D], F32, kind="ExternalOutput").ap()

    with tile.TileContext(nc) as tc, ExitStack() as ctx:
        cons = ctx.enter_context(tc.tile_pool(name="cons", bufs=1))
        xpool = ctx.enter_context(tc.tile_pool(name="xp", bufs=3))
        wef = ctx.enter_context(tc.tile_pool(name="wef", bufs=4))
        h1pool = ctx.enter_context(tc.tile_pool(name="h1p", bufs=1))
        vpool = ctx.enter_context(tc.tile_pool(name="vp", bufs=2))
        scr = ctx.enter_context(tc.tile_pool(name="scr", bufs=2))
        small = ctx.enter_context(tc.tile_pool(name="small", bufs=2))
        psT = ctx.enter_context(tc.tile_pool(name="psT", bufs=2, space="PSUM"))
        ps1 = ctx.enter_context(tc.tile_pool(name="ps1", bufs=2, space="PSUM"))
        ps2a = ctx.enter_context(tc.tile_pool(name="ps2a", bufs=2, space="PSUM"))
        ps2b = ctx.enter_context(tc.tile_pool(name="ps2b", bufs=2, space="PSUM"))

        # ---- pair-0 DMAs first; queue ORDER is the startup critical path
        # (each DMA is ~128 descriptors at ~18ns issue each)
        x_bufs = {}
        x0a = xpool.tile([P, MC, sA0], BF16, tag="X0A", name="x0a")
        nc.gpsimd.dma_start(x0a, x0a_d)
        x0b = xpool.tile([P, MC, Tp0 - sA0], BF16, tag="X0B", name="x0b")
        nc.sync.dma_start(x0b, x0b_d)
        we_bufs = {}
        we0a = wef.tile([P, MC, D // 2], BF16, tag="we0a", name="we0a")
        nc.scalar.dma_start(we0a, we0a_d)
        # rt on the scalar queue (the sync queue kicks off latest); pair-0's
        # drains gate on it, and they gate mm1's PSUM ring reuse
        rt = cons.tile([P, B, DC], F32)
        nc.scalar.dma_start(rt, rt_d)
        we0b = wef.tile([P, MC, D // 2], BF16, tag="we0b", name="we0b")
        nc.sync.dma_start(we0b, we0b_d)
        we_bufs[1] = wef.tile([P, MC, D], BF16, tag="wef", name="we1")
        nc.scalar.dma_start(we_bufs[1], w1e_d[1])

        w2q = cons.tile([P, max(FP8K, 1), D], FP8)
        w2b = cons.tile([P, max(BFK, 1), D], BF16)
        if FP8K > 0:
            nc.gpsimd.dma_start(w2q, w2q_d)
        if BFK > 0:
            nc.sync.dma_start(w2b, w2b_d)

        mask_neg = cons.tile([P, NPAIR, TCmax], F32)
        nc.gpsimd.dma_start(mask_neg, mn_d)
        sel = cons.tile([P, NPAIR, 2], BF16)
        nc.gpsimd.dma_start(sel, sel_d)
        ones_sb = cons.tile([P, 1], F32)
        nc.vector.memset(ones_sb, 1.0)
        # ones matrix: partition-broadcasts the softmax sum via one matmul
        ones_mat = cons.tile([P, P], F32)
        nc.vector.memset(ones_mat, 1.0)

        vals_bufs = {}
        vals_bufs[0] = vpool.tile([P, TCp0, D], BF16, tag="vals", name="vals0")
        nc.sync.dma_start(vals_bufs[0], v_ds[0])

        carry = {}

        def emit_attn_role(st, s, role):
            TCp, cb, rb = st["TCp"], st["cb"], st["rb"]
            if rb > 0:
                cols = list(range(0, cb)) if role == 0 else list(range(cb + 1, TCp))
                edge = st["eA"] if role == 0 else st["eB"]
            else:
                cols = list(range(0, cb)) if role == 0 else list(range(cb, TCp))
                edge = None
            sump = st["sumpA"] if role == 0 else st["sumpB"]
            row = 2 * s + role
            # broadcast 1/sum to all partitions (ones-matrix matmul), then
            # pre-scale exp so the attn matmuls produce the FINAL output in
            # PSUM and the result DMAs straight out - no serial 1-partition
            # drain multiplies on the tail
            tot_ps = psT.tile([P, 1], F32, tag="psT", name=f"tot{row}")
            nc.tensor.matmul(tot_ps, ones_mat, sump, start=True, stop=True)
            rec = small.tile([P, 1], F32, tag="rec")
            nc.vector.reciprocal(rec, tot_ps)
            lhs = []
            rhc = []
            if cols:
                exp_s = small.tile([P, len(cols)], BF16, tag=f"exps{role}")
                nc.vector.tensor_scalar_mul(
                    exp_s, st["exp"][:, cols[0]:cols[-1] + 1], rec)
                lhs += [exp_s[:, k:k + 1] for k in range(len(cols))]
                rhc += cols
            if edge is not None:
                edge_s = small.tile([P, 1], BF16, tag=f"edges{role}")
                nc.vector.tensor_scalar_mul(edge_s, edge, rec)
                lhs.append(edge_s)
                rhc.append(cb)
            out_ps = [psT.tile([1, 512], F32, tag="psT", name=f"ops{row}_{h}")
                      for h in range(NH)]
            for h in range(NH):
                for k in range(len(lhs)):
                    nc.tensor.matmul(
                        out_ps[h], lhs[k], st["vals"][:, rhc[k], _ns(h)],
                        start=(k == 0), stop=(k == len(lhs) - 1),
                    )
            # copies split across Scalar/DVE with per-half DMAs: the two
            # halves drain in parallel and each ships as soon as it's copied
            out_sb = small.tile([1, D], F32, tag="osb")
            nc.scalar.copy(out_sb[:, _ns(0)], out_ps[0])
            nc.gpsimd.dma_start(out_d[row:row + 1, _ns(0)], out_sb[:, _ns(0)])
            nc.vector.tensor_copy(out_sb[:, _ns(1)], out_ps[1])
            nc.gpsimd.dma_start(out_d[row:row + 1, _ns(1)], out_sb[:, _ns(1)])

        def emit_attn_pair(s):
            """Combined both-batch attn@values: a 2-column scaled-exp tile
            (col 0 = batch A's weights, col 1 = B's) streams each values
            chunk ONCE for both outputs."""
            st = carry.pop(s)
            TCp, cb, rb = st["TCp"], st["cb"], st["rb"]
            recs = []
            for role in range(2):
                sump = st["sumpA"] if role == 0 else st["sumpB"]
                tot_ps = psT.tile([P, 1], F32, tag="psT", name=f"tot{2*s+role}")
                nc.tensor.matmul(tot_ps, ones_mat, sump, start=True, stop=True)
                rec = small.tile([P, 1], F32, tag=f"rec{role}")
                nc.vector.reciprocal(rec, tot_ps)
                recs.append(rec)
            e2 = small.tile([P, TCp, 2], BF16, tag="e2")
            nc.gpsimd.memset(e2, 0.0)
            b0 = cb + 1 if rb > 0 else cb
            nc.vector.tensor_scalar_mul(e2[:, 0:cb, 0], st["exp"][:, 0:cb], recs[0])
            nc.vector.tensor_scalar_mul(e2[:, b0:TCp, 1], st["exp"][:, b0:TCp], recs[1])
            if rb > 0:
                nc.vector.tensor_scalar_mul(e2[:, cb, 0:1], st["eA"], recs[0])
                nc.vector.tensor_scalar_mul(e2[:, cb, 1:2], st["eB"], recs[1])
            out_ps = [psT.tile([2, 512], F32, tag="psT", name=f"op2{s}_{h}")
                      for h in range(NH)]
            for h in range(NH):
                for c in range(TCp):
                    nc.tensor.matmul(
                        out_ps[h], e2[:, c, :], st["vals"][:, c, _ns(h)],
                        start=(c == 0), stop=(c == TCp - 1),
                    )
            out_sb = small.tile([2, D], F32, tag="osb2")
            for h in range(NH):
                nc.scalar.copy(out_sb[:, _ns(h)], out_ps[h])
            nc.gpsimd.dma_start(out_d[2 * s:2 * s + 2, :], out_sb)

        # score = (pos-acc - neg-acc)/S_W2 + mask*-1e9, for cols [c0, c1)
        pos_g = ([0] if n_pos > 0 else []) + ([2] if n_pos > 512 else [])
        neg_g = ([1] if n_pos < 512 else []) + ([3] if n_pos < D else [])

        def emit_score(acc, s, TCp, c0, c1, tg):
            gsl = [slice(k * TCp + c0, k * TCp + c1) for k in range(4)]
            w = c1 - c0
            diff = small.tile([P, w], F32, tag=f"diff{tg}")
            if len(pos_g) == 2:
                nc.vector.tensor_tensor(diff, acc[:, gsl[0]], acc[:, gsl[2]],
                                        op=OP.add)
            elif len(pos_g) == 1:
                nc.vector.tensor_copy(diff, acc[:, gsl[pos_g[0]]])
            else:
                nc.vector.memset(diff, 0.0)
            for k in neg_g:
                nc.vector.tensor_sub(diff, diff, acc[:, gsl[k]])
            score_in = small.tile([P, w], F32, tag=f"sin{tg}")
            nc.vector.scalar_tensor_tensor(
                score_in, in0=diff, scalar=1.0 / S_W2, in1=mask_neg[:, s, c0:c1],
                op0=OP.mult, op1=OP.add,
            )
            return score_in

        hsplit = [(0, min(n_pos, 512), min(n_pos, 512), 512),
                  (512, max(n_pos, 512), max(n_pos, 512), D)]

        def emit_accums(acc, TCp, t, h, ps):
            p0, p1, n0, n1 = hsplit[h]
            dump = scr.tile([P, 512], BF16, tag=f"dump{h}")
            if p1 > p0:
                dst = acc[:, 2 * h * TCp + t:2 * h * TCp + t + 1]
                if h == 0:
                    nc.vector.tensor_scalar(
                        dump[:, 0:p1 - p0], ps[:, p0 - 512 * h:p1 - 512 * h],
                        0.0, 0.0, op0=OP.max, op1=OP.add, accum_out=dst)
                else:
                    nc.scalar.activation(
                        dump[:, 0:p1 - p0], ps[:, p0 - 512 * h:p1 - 512 * h],
                        AF.Relu, accum_out=dst)
            if n1 > n0:
                dst = acc[:, (2 * h + 1) * TCp + t:(2 * h + 1) * TCp + t + 1]
                if h == 0:
                    nc.vector.tensor_scalar(
                        dump[:, 512 - (n1 - n0):512], ps[:, n0 - 512 * h:n1 - 512 * h],
                        0.0, 0.0, op0=OP.max, op1=OP.add, accum_out=dst)
                else:
                    nc.scalar.activation(
                        dump[:, 512 - (n1 - n0):512], ps[:, n0 - 512 * h:n1 - 512 * h],
                        AF.Relu, accum_out=dst)

        for s in range(NPAIR):
            LA, LB, L2, TCp, cb, rb = geo[s]
            Tp = TCp * P
            last = (s == NPAIR - 1)

            # prefetch next pair's X / W1eff pair
            if s + 1 < NPAIR:
                Tpn = geo[s + 1][3] * P
                x_bufs[s + 1] = xpool.tile([P, MC, Tpn], BF16, tag="X", name=f"x{s+1}")
                nc.gpsimd.dma_start(x_bufs[s + 1], x_ds[s + 1])
                we_bufs[2 * s + 2] = wef.tile([P, MC, D], BF16, tag="wef",
                                              name=f"we{2*s+2}")
                nc.scalar.dma_start(we_bufs[2 * s + 2], w1e_d[2 * s + 2])
                we_bufs[2 * s + 3] = wef.tile([P, MC, D], BF16, tag="wef",
                                              name=f"we{2*s+3}")
                nc.scalar.dma_start(we_bufs[2 * s + 3], w1e_d[2 * s + 3])

            if s == 0:
                def we_ap(role, c, j):
                    if role == 1:
                        return we_bufs[1][:, c, j * P:(j + 1) * P]
                    return (we0a[:, c, j * P:(j + 1) * P] if j < DC // 2
                            else we0b[:, c, (j - DC // 2) * P:(j - DC // 2 + 1) * P])

                def x_ap(c, s0, s1):
                    return (x0a[:, c, s0:s1] if s0 < sA0
                            else x0b[:, c, s0 - sA0:s1 - sA0])
            else:
                x_t = x_bufs.pop(s)
                weA = we_bufs.pop(2 * s)
                weB = we_bufs.pop(2 * s + 1)

                def we_ap(role, c, j, weA=weA, weB=weB):
                    w = weB if role else weA
                    return w[:, c, j * P:(j + 1) * P]

                def x_ap(c, s0, s1, x_t=x_t):
                    return x_t[:, c, s0:s1]

            # mm1 for both batches of the pair into one packed H1.
            # Drains spread over three engines: the first units + bf16 chunks
            # on Scalar, the rest alternating DVE / GpSimd (Pool) so no single
            # engine's in-order queue stalls the PE's ps1 ring.
            h1q = h1pool.tile([P, max(FP8K, 1), Tp], FP8, tag="H1Q")
            h1b = h1pool.tile([P, max(BFK, 1), Tp], BF16, tag="H1B")
            if L2 < Tp:
                # global pad tail: give it finite h1 so mm2 never reads
                # uninitialized SBUF (fp8/bf16 garbage can be NaN)
                nc.gpsimd.memset(h1q[:, :, L2:Tp], 0.0)
                nc.gpsimd.memset(h1b[:, :, L2:Tp], 0.0)
            mm1_pools = [(ps1, "mm1"), (ps2a, "mm20"), (ps2b, "mm21")]
            unit = 0
            for role in range(2):
                rng = _segs(0, LA) if role == 0 else _segs(LA, L2)
                ridx = 2 * s + role
                # segment-OUTER order: the x0b-dependent tail segment's units
                # come after ~6us of x0a-only work, hiding its DMA latency
                for (s0, s1) in rng:
                    for j in range(DC):
                        pool, ptag = mm1_pools[unit % 3]
                        ps = pool.tile([P, s1 - s0], F32, tag=ptag)
                        for c in range(MC):
                            nc.tensor.matmul(
                                ps, we_ap(role, c, j), x_ap(c, s0, s1),
                                start=(c == 0), stop=(c == MC - 1),
                            )
                        dst = (h1q[:, j, s0:s1] if j < FP8K
                               else h1b[:, j - FP8K, s0:s1])
                        # pair 0: scalar is otherwise idle, so alternate
                        # drains scalar/DVE and the PSUM ring never waits on
                        # one engine; later pairs: scalar handles the first
                        # units (DVE still holds the prior pair's backlog)
                        # plus the bf16 chunks
                        to_scalar = (unit % 2 == 1 if s == 0
                                     else unit < 4)
                        if j >= FP8K or to_scalar:
                            nc.scalar.activation(
                                dst, ps, AF.Relu, bias=rt[:, ridx, j:j + 1],
                            )
                        else:
                            nc.vector.tensor_scalar(
                                dst, ps, rt[:, ridx, j:j + 1], 0.0,
                                op0=OP.add, op1=OP.max,
                            )
                        unit += 1

            if s > 0:
                emit_attn_pair(s - 1)
            if s + 1 < NPAIR:
                TCpn = geo[s + 1][3]
                vals_bufs[s + 1] = vpool.tile([P, TCpn, D], BF16, tag="vals",
                                              name=f"vals{s+1}")
                vq = nc.sync if (s % 2 == 0) else nc.gpsimd
                vq.dma_start(vals_bufs[s + 1], v_ds[s + 1])

            # mm2 (batch-agnostic over packed chunks) + relu-accum scores
            acc = small.tile([P, 4 * TCp], F32, tag="acc")
            exp_str = small.tile([P, TCp], BF16, tag="exps")
            sumpA = small.tile([P, 1], F32, tag="sumpA")
            sumpB = small.tile([P, 1], F32, tag="sumpB")
            eA = eB = None
            sumpA2, sumpB2 = sumpA, sumpB

            def emit_A_phase():
                """Score+exp+sum for batch A's region [0, cb(+1)); on the last
                pair this is emitted mid-mm2 so the chain overlaps the PE."""
                nonlocal eA, eB, sumpA2
                if rb > 0:
                    sc = emit_score(acc, s, TCp, 0, cb + 1, "A")
                    nc.scalar.activation(exp_str[:, 0:cb], sc[:, 0:cb],
                                         AF.Exp, accum_out=sumpA)
                    nc.scalar.activation(exp_str[:, cb:cb + 1], sc[:, cb:cb + 1],
                                         AF.Exp)
                    eA = small.tile([P, 1], BF16, tag="eA")
                    eB = small.tile([P, 1], BF16, tag="eB")
                    nc.vector.tensor_tensor(eA, exp_str[:, cb:cb + 1],
                                            sel[:, s, 0:1], op=OP.mult)
                    nc.vector.tensor_tensor(eB, exp_str[:, cb:cb + 1],
                                            sel[:, s, 1:2], op=OP.mult)
                    sumpA2 = small.tile([P, 1], F32, tag="sumpA2")
                    nc.vector.tensor_tensor(sumpA2, sumpA, eA, op=OP.add)
                else:
                    sc = emit_score(acc, s, TCp, 0, cb, "A")
                    nc.scalar.activation(exp_str[:, 0:cb], sc, AF.Exp,
                                         accum_out=sumpA)

            def emit_B_phase():
                nonlocal sumpB2
                b0 = cb + 1 if rb > 0 else cb
                sc = emit_score(acc, s, TCp, b0, TCp, "B")
                nc.scalar.activation(exp_str[:, b0:TCp], sc, AF.Exp,
                                     accum_out=sumpB)
                if rb > 0:
                    sumpB2 = small.tile([P, 1], F32, tag="sumpB2")
                    nc.vector.tensor_tensor(sumpB2, sumpB, eB, op=OP.add)

            for t in range(TCp):
                tsl = slice(t * P, (t + 1) * P)
                for h in range(NH):
                    ps = (ps2a if h == 0 else ps2b).tile([P, 512], F32, tag=f"mm2{h}")
                    first = True
                    for cp in range(FP8K // 2):
                        nc.tensor.matmul(
                            ps, h1q[:, 2 * cp:2 * cp + 2, tsl],
                            w2q[:, 2 * cp:2 * cp + 2, _ns(h)],
                            start=first, stop=(BFK == 0 and cp == FP8K // 2 - 1),
                            perf_mode=DR,
                        )
                        first = False
                    for cbk in range(BFK):
                        nc.tensor.matmul(
                            ps, h1b[:, cbk, tsl], w2b[:, cbk, _ns(h)],
                            start=first, stop=(cbk == BFK - 1),
                        )
                        first = False
                    emit_accums(acc, TCp, t, h, ps)
                if last and t == cb:
                    emit_A_phase()

            if not last:
                emit_A_phase()
            emit_B_phase()

            st = {"exp": exp_str, "eA": eA, "eB": eB,
                  "sumpA": sumpA2, "sumpB": sumpB2,
                  "vals": vals_bufs.pop(s), "TCp": TCp, "cb": cb, "rb": rb}
            if last:
                emit_attn_role(st, s, 0)
                emit_attn_role(st, s, 1)
            else:
                carry[s] = st

    nc.compile()
    return nc


def _get_built(key):
    if key not in _built:
        _built[key] = _build(key[0], key[1])
    return _built[key]


N_CORES = 8


def prep(query, keys, values, mask, W1, b1, W2, b2, w_score, b_score=None):
    """Host-side pairing + packing + shard + weight fold/cast.

    Returns (build_key, in_maps, perm) where perm[core][row] = global batch."""
    import ml_dtypes

    bf = ml_dtypes.bfloat16
    NB = N_CORES * B
    query = np.ascontiguousarray(np.asarray(query, dtype=np.float32).reshape(NB, M))
    keys = np.asarray(keys, dtype=np.float32).reshape(NB, T, M)
    values = np.asarray(values, dtype=np.float32).reshape(NB, T, D)
    mask = np.asarray(mask, dtype=np.float32).reshape(NB, T)
    W1 = np.asarray(W1, dtype=np.float32)
    b1 = np.asarray(b1, dtype=np.float32)
    W2 = np.asarray(W2, dtype=np.float32)
    w = np.asarray(w_score, dtype=np.float32).reshape(D)

    real = mask < 0.5
    counts = real.sum(axis=1).astype(np.int64)
    order = np.argsort(-counts, kind="stable")

    # slot s pairs rank-group s (largest counts) with rank-group 7-s
    params = []
    perm = [[0] * B for _ in range(N_CORES)]
    for s in range(NPAIR):
        ga = order[8 * s:8 * s + 8]
        gb = order[8 * (7 - s):8 * (7 - s) + 8]
        LA = max(int(counts[ga].max()), P + 1)   # keep boundary off edges
        LB = max(int(counts[gb].max()), P)
        params.append((LA, LB))
        for c in range(N_CORES):
            perm[c][2 * s] = int(ga[c])
            perm[c][2 * s + 1] = int(gb[c])

    # weight folding + host-side rt bias + per-batch effective weights
    W1qc = W1[0:M] + W1[2 * M:3 * M]
    rt_full = query @ W1qc + b1[None, :]
    W1bc = W1[M:2 * M] - W1[2 * M:3 * M]
    W1d = W1[3 * M:4 * M]
    w1eff_all = (W1bc[None, :, :] + query[:, :, None] * W1d[None, :, :]).astype(bf)

    perm_w = np.concatenate([np.where(w > 0)[0], np.where(w <= 0)[0]])
    n_pos = int((w > 0).sum())
    W2F = W2[:, perm_w] * np.abs(w)[perm_w][None, :] * S_W2
    shared = {}
    if FP8K > 0:
        shared["W2Q"] = np.ascontiguousarray(
            W2F[0:FP8K * P].astype(ml_dtypes.float8_e4m3)
            .reshape(FP8K, P, D).transpose(1, 0, 2))
    if BFK > 0:
        shared["W2B"] = np.ascontiguousarray(
            W2F[FP8K * P:D].astype(bf).reshape(BFK, P, D).transpose(1, 0, 2))

    TCmax = max(-(-(LA + LB) // P) for (LA, LB) in params)
    TCp0 = -(-(params[0][0] + params[0][1]) // P)
    sA0 = min(512, TCp0 * P)
    # SEL is identical across cores: depends only on rb per slot
    sel = np.zeros((P, NPAIR, 2), dtype=np.float32)
    for s, (LA, LB) in enumerate(params):
        rb = LA % P
        if rb > 0:
            sel[:rb, s, 0] = 1.0
            sel[rb:, s, 1] = 1.0
    sel = sel.astype(bf)

    in_maps = [dict(shared) for _ in range(N_CORES)]
    rt_all = np.zeros((N_CORES, P, B, DC), dtype=np.float32)
    mn_all = np.zeros((N_CORES, P, NPAIR, TCmax), dtype=np.float32)
    for s, (LA, LB) in enumerate(params):
        TCp = -(-(LA + LB) // P)
        Tp = TCp * P
        for c in range(N_CORES):
            ga = perm[c][2 * s]
            gb = perm[c][2 * s + 1]
            cA = int(counts[ga])
            cB = int(counts[gb])
            xs = np.zeros((Tp, M), dtype=np.float32)
            vs = np.zeros((Tp, D), dtype=np.float32)
            mk = np.ones((Tp,), dtype=np.float32)
            ia = np.nonzero(real[ga])[0]
            ib = np.nonzero(real[gb])[0]
            xs[0:cA] = keys[ga, ia]
            vs[0:cA] = values[ga, ia]
            mk[0:cA] = 0.0
            xs[LA:LA + cB] = keys[gb, ib]
            vs[LA:LA + cB] = values[gb, ib]
            mk[LA:LA + cB] = 0.0
            # SBUF layouts: X -> [P, MC, Tp], V -> [P, TCp, D]
            xp = xs.T.astype(bf).reshape(MC, P, Tp).transpose(1, 0, 2)
            vp = vs.astype(bf).reshape(TCp, P, D).transpose(1, 0, 2)
            if s == 0:
                in_maps[c]["X0A"] = np.ascontiguousarray(xp[:, :, 0:sA0])
                in_maps[c]["X0B"] = np.ascontiguousarray(xp[:, :, sA0:])
            else:
                in_maps[c][f"X{s}"] = np.ascontiguousarray(xp)
            in_maps[c][f"V{s}"] = np.ascontiguousarray(vp)
            mn_all[c, :, s, 0:TCp] = mk.reshape(TCp, P).T * NEG
            for role, gg in ((0, ga), (1, gb)):
                rt_all[c, :, 2 * s + role] = rt_full[gg].reshape(DC, P).T
    for c in range(N_CORES):
        # W1eff -> [B, P, MC, D]; batch 0 additionally split in half
        wb = np.stack([w1eff_all[perm[c][r]] for r in range(B)])  # [B, M, D]
        wp = np.ascontiguousarray(
            wb.reshape(B, MC, P, D).transpose(0, 2, 1, 3))        # [B, P, MC, D]
        in_maps[c]["W1EFF"] = wp
        in_maps[c]["WE0A"] = np.ascontiguousarray(wp[0][:, :, 0:D // 2])
        in_maps[c]["WE0B"] = np.ascontiguousarray(wp[0][:, :, D // 2:])
        in_maps[c]["RT"] = np.ascontiguousarray(rt_all[c])
        in_maps[c]["MASKN"] = np.ascontiguousarray(mn_all[c])
        in_maps[c]["SEL"] = sel

    return (n_pos, tuple(params)), in_maps, perm


def gather_out(results, perm):
    out = np.zeros((N_CORES * B, 1, D), dtype=np.float32)
    for c in range(N_CORES):
        o = results[c]["out"]
        for r in range(B):
            out[perm[c][r], 0, :] = o[r]
    return out


def kernel(query, keys, values, mask, W1, b1, W2, b2, w_score, b_score):
    """Full-input entry point: shards over 8 NeuronCores, returns [64, 1, D]."""
    from concourse.bass_utils import run_bass_kernel_spmd

    build_key, in_maps, perm = prep(query, keys, values, mask, W1, b1, W2, b2, w_score)
    nc = _get_built(build_key)
    res = run_bass_kernel_spmd(nc, in_maps, core_ids=list(range(N_CORES)))
    return gather_out(res.results, perm)


# revision 38
# speedup vs baseline: 1.1592x; 1.1592x over previous
"""DIN attention layer kernel for Trainium2 - batch-PAIR token packing.

Per batch b (reference): att=[q,k,q-k,q*k]; h1=relu(att@W1+b1);
h2=relu(h1@W2+b2); s=h2@w_score; attn=softmax(s + mask*-1e9);
out=attn@values.

Optimizations:
  * Host token compaction: masked tokens (mask==1 -> -1e9 logit -> attn
    weight exactly 0 in fp32) are dropped on the host; only the ~50%
    real tokens reach the device.
  * Batch-pair packing: two batches' real tokens share one packed token
    axis of ceil((LA+LB)/128) chunks (LA/LB = per-slot maxima over
    cores, baked). mm2 + score work are batch-agnostic; pad waste drops
    from ~132 tokens/batch to ~64/pair (40 -> ~33 chunks/core). The
    A|B boundary chunk is handled with 0/1 column selectors (SEL) so no
    partition-offset matmuls are needed; per-batch softmax sums come
    from splitting the Exp activation at the boundary + a DVE add.
  * Concat-matmul reassociation: att@W1 = q@(W1a+W1c) [host, folds into
    the rt bias] + k@[(W1b-W1c) + diag(q)W1d] [device mm1, contraction
    256]. The per-batch W1eff is computed on the host and DMA'd.
  * All tensors arrive in exact SBUF tile layout ([P, ...] contiguous
    per partition) so every DMA is a handful of multi-KB descriptors.
  * mm2 in transposed-output form; score falls out of the PSUM drain
    via relu-accumulate with W2 columns pre-permuted by sign(w_score)
    and pre-scaled by |w_score|. Each 512-col half has its own PSUM
    pool; half 0 accumulates on the DVE, half 1 on Scalar.
  * mm2 hybrid precision: 6 of 8 contraction chunks fp8e4 DoubleRow
    (2x), 2 bf16; rel err ~1.77e-2 vs the 2e-2 gate.
  * Softmax without max-subtraction; attn@values accumulated per 128-
    token chunk with exp as lhsT (scores land partition-striped free).
  * Software pipelining: pair s emits mm1(s), attn(s-1), mm2(s); the
    last pair splits its softmax so batch A's attn chain overlaps the
    tail chunks of mm2.
"""

import os
import numpy as np

P = 128
B = 8          # batches per core
NPAIR = 4      # batch pairs per core
T = 1024       # tokens (full, pre-compaction)
M = 256        # key feature dim
D = 1024       # hidden dim
MC = M // P    # key-feature chunks (2)
DC = D // P    # hidden chunks (8)
NH = 2         # free-dim halves of 512
NEG = -1.0e9
S_W2 = 512.0   # pre-scale on W2'' (keeps fp8 path out of denormals)
FP8K = int(os.environ.get("DIN_FP8K", "6"))   # mm2 contraction chunks in fp8
BFK = DC - FP8K

_built = {}


def _ns(h):
    return slice(h * 512, (h + 1) * 512)


def _segs(a, b):
    """Split [a, b) into free-dim segments of <= 512."""
    return [(s, min(s + 512, b)) for s in range(a, b, 512)]


def _build(n_pos, params):
    import concourse.bass as bass
    import concourse.bacc as bacc
    import concourse.mybir as mybir
    import concourse.tile as tile
    from contextlib import ExitStack

    F32 = mybir.dt.float32
    BF16 = mybir.dt.bfloat16
    FP8 = mybir.dt.float8e4
    AF = mybir.ActivationFunctionType
    OP = mybir.AluOpType
    DR = mybir.MatmulPerfMode.DoubleRow

    geo = []
    for (LA, LB) in params:
        L2 = LA + LB
        TCp = -(-L2 // P)
        cb, rb = divmod(LA, P)
        assert cb >= 1 and TCp - cb >= 2, (LA, LB)
        geo.append((LA, LB, L2, TCp, cb, rb))
    TCmax = max(g[3] for g in geo)
    TCp0 = geo[0][3]
    Tp0 = TCp0 * P
    sA0 = min(512, Tp0)

    nc = bacc.Bacc("TRN2")
    # pair 0's X / W1eff arrive as split tensors for a fast start
    x0a_d = nc.dram_tensor("X0A", [P, MC, sA0], BF16, kind="ExternalInput").ap()
    x0b_d = nc.dram_tensor("X0B", [P, MC, Tp0 - sA0], BF16,
                           kind="ExternalInput").ap()
    x_ds = [None] + [nc.dram_tensor(f"X{s}", [P, MC, geo[s][3] * P], BF16,
                                    kind="ExternalInput").ap()
                     for s in range(1, NPAIR)]
    v_ds = [nc.dram_tensor(f"V{s}", [P, geo[s][3], D], BF16,
                           kind="ExternalInput").ap() for s in range(NPAIR)]
    rt_d = nc.dram_tensor("RT", [P, B, DC], F32, kind="ExternalInput").ap()
    mn_d = nc.dram_tensor("MASKN", [P, NPAIR, TCmax], F32, kind="ExternalInput").ap()
    sel_d = nc.dram_tensor("SEL", [P, NPAIR, 2], BF16, kind="ExternalInput").ap()
    we0a_d = nc.dram_tensor("WE0A", [P, MC, D // 2], BF16, kind="ExternalInput").ap()
    we0b_d = nc.dram_tensor("WE0B", [P, MC, D // 2], BF16, kind="ExternalInput").ap()
    w1e_d = nc.dram_tensor("W1EFF", [B, P, MC, D], BF16, kind="ExternalInput").ap()
    w2q_d = (nc.dram_tensor("W2Q", [P, FP8K, D], FP8, kind="ExternalInput").ap()
             if FP8K > 0 else None)
    w2b_d = (nc.dram_tensor("W2B", [P, BFK, D], BF16, kind="ExternalInput").ap()
             if BFK > 0 else None)
    out_d = nc.dram_tensor("out", [B, D], F32, kind="ExternalOutput").ap()

    with tile.TileContext(nc) as tc, ExitStack() as ctx:
        cons = ctx.enter_context(tc.tile_pool(name="cons", bufs=1))
        xpool = ctx.enter_context(tc.tile_pool(name="xp", bufs=3))
        wef = ctx.enter_context(tc.tile_pool(name="wef", bufs=4))
        h1pool = ctx.enter_context(tc.tile_pool(name="h1p", bufs=1))
        vpool = ctx.enter_context(tc.tile_pool(name="vp", bufs=2))
        scr = ctx.enter_context(tc.tile_pool(name="scr", bufs=2))
        small = ctx.enter_context(tc.tile_pool(name="small", bufs=2))
        psT = ctx.enter_context(tc.tile_pool(name="psT", bufs=2, space="PSUM"))
        ps1 = ctx.enter_context(tc.tile_pool(name="ps1", bufs=2, space="PSUM"))
        ps2a = ctx.enter_context(tc.tile_pool(name="ps2a", bufs=2, space="PSUM"))
        ps2b = ctx.enter_context(tc.tile_pool(name="ps2b", bufs=2, space="PSUM"))

        # ---- pair-0 DMAs first; queue ORDER is the startup critical path
        # (each DMA is ~128 descriptors at ~18ns issue each)
        x_bufs = {}
        x0a = xpool.tile([P, MC, sA0], BF16, tag="X0A", name="x0a")
        nc.gpsimd.dma_start(x0a, x0a_d)
        x0b = xpool.tile([P, MC, Tp0 - sA0], BF16, tag="X0B", name="x0b")
        nc.sync.dma_start(x0b, x0b_d)
        we_bufs = {}
        we0a = wef.tile([P, MC, D // 2], BF16, tag="we0a", name="we0a")
        nc.scalar.dma_start(we0a, we0a_d)
        rt = cons.tile([P, B, DC], F32)
        nc.scalar.dma_start(rt, rt_d)
        we0b = wef.tile([P, MC, D // 2], BF16, tag="we0b", name="we0b")
        nc.sync.dma_start(we0b, we0b_d)
        we_bufs[1] = wef.tile([P, MC, D], BF16, tag="wef", name="we1")
        nc.scalar.dma_start(we_bufs[1], w1e_d[1])

        w2q = cons.tile([P, max(FP8K, 1), D], FP8)
        w2b = cons.tile([P, max(BFK, 1), D], BF16)
        if FP8K > 0:
            nc.gpsimd.dma_start(w2q, w2q_d)
        if BFK > 0:
            nc.sync.dma_start(w2b, w2b_d)

        mask_neg = cons.tile([P, NPAIR, TCmax], F32)
        nc.gpsimd.dma_start(mask_neg, mn_d)
        sel = cons.tile([P, NPAIR, 2], BF16)
        nc.gpsimd.dma_start(sel, sel_d)
        ones_sb = cons.tile([P, 1], F32)
        nc.vector.memset(ones_sb, 1.0)
        # ones matrix: partition-broadcasts the softmax sum via one matmul
        ones_mat = cons.tile([P, P], F32)
        nc.vector.memset(ones_mat, 1.0)

        vals_bufs = {}
        vals_bufs[0] = vpool.tile([P, TCp0, D], BF16, tag="vals", name="vals0")
        nc.sync.dma_start(vals_bufs[0], v_ds[0])

        carry = {}

        def emit_attn_role(st, s, role):
            TCp, cb, rb = st["TCp"], st["cb"], st["rb"]
            if rb > 0:
                cols = list(range(0, cb)) if role == 0 else list(range(cb + 1, TCp))
                edge = st["eA"] if role == 0 else st["eB"]
            else:
                cols = list(range(0, cb)) if role == 0 else list(range(cb, TCp))
                edge = None
            sump = st["sumpA"] if role == 0 else st["sumpB"]
            row = 2 * s + role
            # broadcast 1/sum to all partitions (ones-matrix matmul), then
            # pre-scale exp so the attn matmuls produce the FINAL output in
            # PSUM and the result DMAs straight out - no serial 1-partition
            # drain multiplies on the tail
            tot_ps = psT.tile([P, 1], F32, tag="psT", name=f"tot{row}")
            nc.tensor.matmul(tot_ps, ones_mat, sump, start=True, stop=True)
            rec = small.tile([P, 1], F32, tag="rec")
            nc.vector.reciprocal(rec, tot_ps)
            lhs = []
            rhc = []
            if cols:
                exp_s = small.tile([P, len(cols)], BF16, tag=f"exps{role}")
                nc.vector.tensor_scalar_mul(
                    exp_s, st["exp"][:, cols[0]:cols[-1] + 1], rec)
                lhs += [exp_s[:, k:k + 1] for k in range(len(cols))]
                rhc += cols
            if edge is not None:
                edge_s = small.tile([P, 1], BF16, tag=f"edges{role}")
                nc.vector.tensor_scalar_mul(edge_s, edge, rec)
                lhs.append(edge_s)
                rhc.append(cb)
            out_ps = [psT.tile([1, 512], F32, tag="psT", name=f"ops{row}_{h}")
                      for h in range(NH)]
            for h in range(NH):
                for k in range(len(lhs)):
                    nc.tensor.matmul(
                        out_ps[h], lhs[k], st["vals"][:, rhc[k], _ns(h)],
                        start=(k == 0), stop=(k == len(lhs) - 1),
                    )
            # copies split across Scalar/DVE with per-half DMAs: the halves
            # drain in parallel and each ships as soon as it's copied
            out_sb = small.tile([1, D], F32, tag="osb")
            nc.scalar.copy(out_sb[:, _ns(0)], out_ps[0])
            nc.gpsimd.dma_start(out_d[row:row + 1, _ns(0)], out_sb[:, _ns(0)])
            nc.vector.tensor_copy(out_sb[:, _ns(1)], out_ps[1])
            nc.gpsimd.dma_start(out_d[row:row + 1, _ns(1)], out_sb[:, _ns(1)])

        def emit_attn_pair(s):
            """Combined both-batch attn@values: a 2-column scaled-exp tile
            (col 0 = batch A's weights, col 1 = B's) streams each values
            chunk ONCE for both outputs."""
            st = carry.pop(s)
            TCp, cb, rb = st["TCp"], st["cb"], st["rb"]
            recs = []
            for role in range(2):
                sump = st["sumpA"] if role == 0 else st["sumpB"]
                tot_ps = psT.tile([P, 1], F32, tag="psT", name=f"tot{2*s+role}")
                nc.tensor.matmul(tot_ps, ones_mat, sump, start=True, stop=True)
                rec = small.tile([P, 1], F32, tag=f"rec{role}")
                nc.vector.reciprocal(rec, tot_ps)
                recs.append(rec)
            e2 = small.tile([P, TCp, 2], BF16, tag="e2")
            nc.gpsimd.memset(e2, 0.0)
            b0 = cb + 1 if rb > 0 else cb
            nc.vector.tensor_scalar_mul(e2[:, 0:cb, 0], st["exp"][:, 0:cb], recs[0])
            nc.vector.tensor_scalar_mul(e2[:, b0:TCp, 1], st["exp"][:, b0:TCp], recs[1])
            if rb > 0:
                nc.vector.tensor_scalar_mul(e2[:, cb, 0:1], st["eA"], recs[0])
                nc.vector.tensor_scalar_mul(e2[:, cb, 1:2], st["eB"], recs[1])
            out_ps = [psT.tile([2, 512], F32, tag="psT", name=f"op2{s}_{h}")
                      for h in range(NH)]
            for h in range(NH):
                for c in range(TCp):
                    nc.tensor.matmul(
                        out_ps[h], e2[:, c, :], st["vals"][:, c, _ns(h)],
                        start=(c == 0), stop=(c == TCp - 1),
                    )
            out_sb = small.tile([2, D], F32, tag="osb2")
            for h in range(NH):
                nc.scalar.copy(out_sb[:, _ns(h)], out_ps[h])
            nc.gpsimd.dma_start(out_d[2 * s:2 * s + 2, :], out_sb)

        # score = (pos-acc - neg-acc)/S_W2 + mask*-1e9, for cols [c0, c1)
        pos_g = ([0] if n_pos > 0 else []) + ([2] if n_pos > 512 else [])
        neg_g = ([1] if n_pos < 512 else []) + ([3] if n_pos < D else [])

        def emit_score(acc, s, TCp, c0, c1, tg):
            gsl = [slice(k * TCp + c0, k * TCp + c1) for k in range(4)]
            w = c1 - c0
            diff = small.tile([P, w], F32, tag=f"diff{tg}")
            if len(pos_g) == 2:
                nc.vector.tensor_tensor(diff, acc[:, gsl[0]], acc[:, gsl[2]],
                                        op=OP.add)
            elif len(pos_g) == 1:
                nc.vector.tensor_copy(diff, acc[:, gsl[pos_g[0]]])
            else:
                nc.vector.memset(diff, 0.0)
            for k in neg_g:
                nc.vector.tensor_sub(diff, diff, acc[:, gsl[k]])
            score_in = small.tile([P, w], F32, tag=f"sin{tg}")
            nc.vector.scalar_tensor_tensor(
                score_in, in0=diff, scalar=1.0 / S_W2, in1=mask_neg[:, s, c0:c1],
                op0=OP.mult, op1=OP.add,
            )
            return score_in

        hsplit = [(0, min(n_pos, 512), min(n_pos, 512), 512),
                  (512, max(n_pos, 512), max(n_pos, 512), D)]

        def emit_accums(acc, TCp, t, h, ps):
            p0, p1, n0, n1 = hsplit[h]
            dump = scr.tile([P, 512], BF16, tag=f"dump{h}")
            if p1 > p0:
                dst = acc[:, 2 * h * TCp + t:2 * h * TCp + t + 1]
                if h == 0:
                    nc.vector.tensor_scalar(
                        dump[:, 0:p1 - p0], ps[:, p0 - 512 * h:p1 - 512 * h],
                        0.0, 0.0, op0=OP.max, op1=OP.add, accum_out=dst)
                else:
                    nc.scalar.activation(
                        dump[:, 0:p1 - p0], ps[:, p0 - 512 * h:p1 - 512 * h],
                        AF.Relu, accum_out=dst)
            if n1 > n0:
                dst = acc[:, (2 * h + 1) * TCp + t:(2 * h + 1) * TCp + t + 1]
                if h == 0:
                    nc.vector.tensor_scalar(
                        dump[:, 512 - (n1 - n0):512], ps[:, n0 - 512 * h:n1 - 512 * h],
                        0.0, 0.0, op0=OP.max, op1=OP.add, accum_out=dst)
                else:
                    nc.scalar.activation(
                        dump[:, 512 - (n1 - n0):512], ps[:, n0 - 512 * h:n1 - 512 * h],
                        AF.Relu, accum_out=dst)

        for s in range(NPAIR):
            LA, LB, L2, TCp, cb, rb = geo[s]
            Tp = TCp * P
            last = (s == NPAIR - 1)

            # prefetch next pair's X / W1eff pair
            if s + 1 < NPAIR:
                Tpn = geo[s + 1][3] * P
                x_bufs[s + 1] = xpool.tile([P, MC, Tpn], BF16, tag="X", name=f"x{s+1}")
                nc.gpsimd.dma_start(x_bufs[s + 1], x_ds[s + 1])
                we_bufs[2 * s + 2] = wef.tile([P, MC, D], BF16, tag="wef",
                                              name=f"we{2*s+2}")
                nc.scalar.dma_start(we_bufs[2 * s + 2], w1e_d[2 * s + 2])
                we_bufs[2 * s + 3] = wef.tile([P, MC, D], BF16, tag="wef",
                                              name=f"we{2*s+3}")
                nc.scalar.dma_start(we_bufs[2 * s + 3], w1e_d[2 * s + 3])

            if s == 0:
                def we_ap(role, c, j):
                    if role == 1:
                        return we_bufs[1][:, c, j * P:(j + 1) * P]
                    return (we0a[:, c, j * P:(j + 1) * P] if j < DC // 2
                            else we0b[:, c, (j - DC // 2) * P:(j - DC // 2 + 1) * P])

                def x_ap(c, s0, s1):
                    return (x0a[:, c, s0:s1] if s0 < sA0
                            else x0b[:, c, s0 - sA0:s1 - sA0])
            else:
                x_t = x_bufs.pop(s)
                weA = we_bufs.pop(2 * s)
                weB = we_bufs.pop(2 * s + 1)

                def we_ap(role, c, j, weA=weA, weB=weB):
                    w = weB if role else weA
                    return w[:, c, j * P:(j + 1) * P]

                def x_ap(c, s0, s1, x_t=x_t):
                    return x_t[:, c, s0:s1]

            # mm1 for both batches of the pair into one packed H1.
            # Drains spread over three engines: the first units + bf16 chunks
            # on Scalar, the rest alternating DVE / GpSimd (Pool) so no single
            # engine's in-order queue stalls the PE's ps1 ring.
            h1q = h1pool.tile([P, max(FP8K, 1), Tp], FP8, tag="H1Q")
            h1b = h1pool.tile([P, max(BFK, 1), Tp], BF16, tag="H1B")
            if L2 < Tp:
                # global pad tail: give it finite h1 so mm2 never reads
                # uninitialized SBUF (fp8/bf16 garbage can be NaN)
                nc.gpsimd.memset(h1q[:, :, L2:Tp], 0.0)
                nc.gpsimd.memset(h1b[:, :, L2:Tp], 0.0)
            mm1_pools = [(ps1, "mm1"), (ps2a, "mm20"), (ps2b, "mm21")]
            unit = 0
            for role in range(2):
                rng = _segs(0, LA) if role == 0 else _segs(LA, L2)
                ridx = 2 * s + role
                # segment-OUTER order: the x0b-dependent tail segment's units
                # come after ~6us of x0a-only work, hiding its DMA latency
                for (s0, s1) in rng:
                    for j in range(DC):
                        pool, ptag = mm1_pools[unit % 3]
                        ps = pool.tile([P, s1 - s0], F32, tag=ptag)
                        for c in range(MC):
                            nc.tensor.matmul(
                                ps, we_ap(role, c, j), x_ap(c, s0, s1),
                                start=(c == 0), stop=(c == MC - 1),
                            )
                        dst = (h1q[:, j, s0:s1] if j < FP8K
                               else h1b[:, j - FP8K, s0:s1])
                        # pair 0: scalar is otherwise idle, so alternate
                        # drains scalar/DVE and the ps1 ring never waits on
                        # a single engine's backlog; later pairs: scalar
                        # takes the first units + the bf16 chunks
                        to_scalar = (unit % 2 == 1) if s == 0 else (unit < 4)
                        if j >= FP8K or to_scalar:
                            nc.scalar.activation(
                                dst, ps, AF.Relu, bias=rt[:, ridx, j:j + 1],
                            )
                        else:
                            nc.vector.tensor_scalar(
                                dst, ps, rt[:, ridx, j:j + 1], 0.0,
                                op0=OP.add, op1=OP.max,
                            )
                        unit += 1

            if s > 0:
                emit_attn_pair(s - 1)
            if s + 1 < NPAIR:
                TCpn = geo[s + 1][3]
                vals_bufs[s + 1] = vpool.tile([P, TCpn, D], BF16, tag="vals",
                                              name=f"vals{s+1}")
                vq = nc.sync if (s % 2 == 0) else nc.gpsimd
                vq.dma_start(vals_bufs[s + 1], v_ds[s + 1])

            # mm2 (batch-agnostic over packed chunks) + relu-accum scores
            acc = small.tile([P, 4 * TCp], F32, tag="acc")
            exp_str = small.tile([P, TCp], BF16, tag="exps")
            sumpA = small.tile([P, 1], F32, tag="sumpA")
            sumpB = small.tile([P, 1], F32, tag="sumpB")
            eA = eB = None
            sumpA2, sumpB2 = sumpA, sumpB

            def emit_A_phase():
                """Score+exp+sum for batch A's region [0, cb(+1)); on the last
                pair this is emitted mid-mm2 so the chain overlaps the PE."""
                nonlocal eA, eB, sumpA2
                if rb > 0:
                    sc = emit_score(acc, s, TCp, 0, cb + 1, "A")
                    nc.scalar.activation(exp_str[:, 0:cb], sc[:, 0:cb],
                                         AF.Exp, accum_out=sumpA)
                    nc.scalar.activation(exp_str[:, cb:cb + 1], sc[:, cb:cb + 1],
                                         AF.Exp)
                    eA = small.tile([P, 1], BF16, tag="eA")
                    eB = small.tile([P, 1], BF16, tag="eB")
                    nc.vector.tensor_tensor(eA, exp_str[:, cb:cb + 1],
                                            sel[:, s, 0:1], op=OP.mult)
                    nc.vector.tensor_tensor(eB, exp_str[:, cb:cb + 1],
                                            sel[:, s, 1:2], op=OP.mult)
                    sumpA2 = small.tile([P, 1], F32, tag="sumpA2")
                    nc.vector.tensor_tensor(sumpA2, sumpA, eA, op=OP.add)
                else:
                    sc = emit_score(acc, s, TCp, 0, cb, "A")
                    nc.scalar.activation(exp_str[:, 0:cb], sc, AF.Exp,
                                         accum_out=sumpA)

            def emit_B_phase():
                nonlocal sumpB2
                b0 = cb + 1 if rb > 0 else cb
                sc = emit_score(acc, s, TCp, b0, TCp, "B")
                nc.scalar.activation(exp_str[:, b0:TCp], sc, AF.Exp,
                                     accum_out=sumpB)
                if rb > 0:
                    sumpB2 = small.tile([P, 1], F32, tag="sumpB2")
                    nc.vector.tensor_tensor(sumpB2, sumpB, eB, op=OP.add)

            for t in range(TCp):
                tsl = slice(t * P, (t + 1) * P)
                for h in range(NH):
                    ps = (ps2a if h == 0 else ps2b).tile([P, 512], F32, tag=f"mm2{h}")
                    first = True
                    for cp in range(FP8K // 2):
                        nc.tensor.matmul(
                            ps, h1q[:, 2 * cp:2 * cp + 2, tsl],
                            w2q[:, 2 * cp:2 * cp + 2, _ns(h)],
                            start=first, stop=(BFK == 0 and cp == FP8K // 2 - 1),
                            perf_mode=DR,
                        )
                        first = False
                    for cbk in range(BFK):
                        nc.tensor.matmul(
                            ps, h1b[:, cbk, tsl], w2b[:, cbk, _ns(h)],
                            start=first, stop=(cbk == BFK - 1),
                        )
                        first = False
                    emit_accums(acc, TCp, t, h, ps)
                if last and t == cb:
                    emit_A_phase()

            if not last:
                emit_A_phase()
            emit_B_phase()

            st = {"exp": exp_str, "eA": eA, "eB": eB,
                  "sumpA": sumpA2, "sumpB": sumpB2,
                  "vals": vals_bufs.pop(s), "TCp": TCp, "cb": cb, "rb": rb}
            if last:
                emit_attn_role(st, s, 0)
                emit_attn_role(st, s, 1)
            else:
                carry[s] = st

    nc.compile()
    return nc


def _get_built(key):
    if key not in _built:
        _built[key] = _build(key[0], key[1])
    return _built[key]


N_CORES = 8


def prep(query, keys, values, mask, W1, b1, W2, b2, w_score, b_score=None):
    """Host-side pairing + packing + shard + weight fold/cast.

    Returns (build_key, in_maps, perm) where perm[core][row] = global batch."""
    import ml_dtypes

    bf = ml_dtypes.bfloat16
    NB = N_CORES * B
    query = np.ascontiguousarray(np.asarray(query, dtype=np.float32).reshape(NB, M))
    keys = np.asarray(keys, dtype=np.float32).reshape(NB, T, M)
    values = np.asarray(values, dtype=np.float32).reshape(NB, T, D)
    mask = np.asarray(mask, dtype=np.float32).reshape(NB, T)
    W1 = np.asarray(W1, dtype=np.float32)
    b1 = np.asarray(b1, dtype=np.float32)
    W2 = np.asarray(W2, dtype=np.float32)
    w = np.asarray(w_score, dtype=np.float32).reshape(D)

    real = mask < 0.5
    counts = real.sum(axis=1).astype(np.int64)
    order = np.argsort(-counts, kind="stable")

    # slot s pairs rank-group s (largest counts) with rank-group 7-s
    params = []
    perm = [[0] * B for _ in range(N_CORES)]
    for s in range(NPAIR):
        ga = order[8 * s:8 * s + 8]
        gb = order[8 * (7 - s):8 * (7 - s) + 8]
        LA = max(int(counts[ga].max()), P + 1)   # keep boundary off edges
        LB = max(int(counts[gb].max()), P)
        params.append((LA, LB))
        for c in range(N_CORES):
            perm[c][2 * s] = int(ga[c])
            perm[c][2 * s + 1] = int(gb[c])

    # weight folding + host-side rt bias + per-batch effective weights
    W1qc = W1[0:M] + W1[2 * M:3 * M]
    rt_full = query @ W1qc + b1[None, :]
    W1bc = W1[M:2 * M] - W1[2 * M:3 * M]
    W1d = W1[3 * M:4 * M]
    w1eff_all = (W1bc[None, :, :] + query[:, :, None] * W1d[None, :, :]).astype(bf)

    perm_w = np.concatenate([np.where(w > 0)[0], np.where(w <= 0)[0]])
    n_pos = int((w > 0).sum())
    W2F = W2[:, perm_w] * np.abs(w)[perm_w][None, :] * S_W2
    shared = {}
    if FP8K > 0:
        shared["W2Q"] = np.ascontiguousarray(
            W2F[0:FP8K * P].astype(ml_dtypes.float8_e4m3)
            .reshape(FP8K, P, D).transpose(1, 0, 2))
    if BFK > 0:
        shared["W2B"] = np.ascontiguousarray(
            W2F[FP8K * P:D].astype(bf).reshape(BFK, P, D).transpose(1, 0, 2))

    TCmax = max(-(-(LA + LB) // P) for (LA, LB) in params)
    TCp0 = -(-(params[0][0] + params[0][1]) // P)
    sA0 = min(512, TCp0 * P)
    # SEL is identical across cores: depends only on rb per slot
    sel = np.zeros((P, NPAIR, 2), dtype=np.float32)
    for s, (LA, LB) in enumerate(params):
        rb = LA % P
        if rb > 0:
            sel[:rb, s, 0] = 1.0
            sel[rb:, s, 1] = 1.0
    sel = sel.astype(bf)

    in_maps = [dict(shared) for _ in range(N_CORES)]
    rt_all = np.zeros((N_CORES, P, B, DC), dtype=np.float32)
    mn_all = np.zeros((N_CORES, P, NPAIR, TCmax), dtype=np.float32)
    for s, (LA, LB) in enumerate(params):
        TCp = -(-(LA + LB) // P)
        Tp = TCp * P
        for c in range(N_CORES):
            ga = perm[c][2 * s]
            gb = perm[c][2 * s + 1]
            cA = int(counts[ga])
            cB = int(counts[gb])
            xs = np.zeros((Tp, M), dtype=np.float32)
            vs = np.zeros((Tp, D), dtype=np.float32)
            mk = np.ones((Tp,), dtype=np.float32)
            ia = np.nonzero(real[ga])[0]
            ib = np.nonzero(real[gb])[0]
            xs[0:cA] = keys[ga, ia]
            vs[0:cA] = values[ga, ia]
            mk[0:cA] = 0.0
            xs[LA:LA + cB] = keys[gb, ib]
            vs[LA:LA + cB] = values[gb, ib]
            mk[LA:LA + cB] = 0.0
            # SBUF layouts: X -> [P, MC, Tp], V -> [P, TCp, D]
            xp = xs.T.astype(bf).reshape(MC, P, Tp).transpose(1, 0, 2)
            vp = vs.astype(bf).reshape(TCp, P, D).transpose(1, 0, 2)
            if s == 0:
                in_maps[c]["X0A"] = np.ascontiguousarray(xp[:, :, 0:sA0])
                in_maps[c]["X0B"] = np.ascontiguousarray(xp[:, :, sA0:])
            else:
                in_maps[c][f"X{s}"] = np.ascontiguousarray(xp)
            in_maps[c][f"V{s}"] = np.ascontiguousarray(vp)
            mn_all[c, :, s, 0:TCp] = mk.reshape(TCp, P).T * NEG
            for role, gg in ((0, ga), (1, gb)):
                rt_all[c, :, 2 * s + role] = rt_full[gg].reshape(DC, P).T
    for c in range(N_CORES):
        # W1eff -> [B, P, MC, D]; batch 0 additionally split in half
        wb = np.stack([w1eff_all[perm[c][r]] for r in range(B)])  # [B, M, D]
        wp = np.ascontiguousarray(
            wb.reshape(B, MC, P, D).transpose(0, 2, 1, 3))        # [B, P, MC, D]
        in_maps[c]["W1EFF"] = wp
        in_maps[c]["WE0A"] = np.ascontiguousarray(wp[0][:, :, 0:D // 2])
        in_maps[c]["WE0B"] = np.ascontiguousarray(wp[0][:, :, D // 2:])
        in_maps[c]["RT"] = np.ascontiguousarray(rt_all[c])
        in_maps[c]["MASKN"] = np.ascontiguousarray(mn_all[c])
        in_maps[c]["SEL"] = sel

    return (n_pos, tuple(params)), in_maps, perm


def gather_out(results, perm):
    out = np.zeros((N_CORES * B, 1, D), dtype=np.float32)
    for c in range(N_CORES):
        o = results[c]["out"]
        for r in range(B):
            out[perm[c][r], 0, :] = o[r]
    return out


def kernel(query, keys, values, mask, W1, b1, W2, b2, w_score, b_score):
    """Full-input entry point: shards over 8 NeuronCores, returns [64, 1, D]."""
    from concourse.bass_utils import run_bass_kernel_spmd

    build_key, in_maps, perm = prep(query, keys, values, mask, W1, b1, W2, b2, w_score)
    nc = _get_built(build_key)
    res = run_bass_kernel_spmd(nc, in_maps, core_ids=list(range(N_CORES)))
    return gather_out(res.results, perm)
